# revision 7
# baseline (speedup 1.0000x reference)
"""Trainium2 Bass kernel for nn_Data_augV4 (per-image augmentation routing).

Semantics (matches the JAX reference):
  for step in range(2):
      per image b: out = TFS[samples[step][b]](out), TFS =
      [identity, flip_lr(W), flip_ud(H), brightness, contrast, solarize,
       invert, posterize]
  (`prob` does not affect the output.)

Strategy:
  * Pure data parallel: 16 images per core, 8 cores, ONE SPMD program.
  * `samples` is known when kernel() builds the program, so the host plans
    per-image work queues (per primitive kind, per half/round). Queue lengths
    are padded to the max across cores so every core runs the same program;
    per-item image offsets and coefficients are passed as data and read into
    registers (dynamic ds() access patterns).
  * SBUF layout per image: [112 partitions, (c:3, s:2, w:224)]; partition p
    holds rows h=p (s=0) and h=112+p (s=1). All DMAs use ascending strides.
    W-flip is a free-dim w reversal. H-flip = s-swap + partition reversal;
    the partition reversal runs on the tensor engine as J @ X with J the
    112x112 anti-identity (exact for 0/1 weights), and the s-swap/w-reversal
    fold into the PSUM->SBUF evacuation copy on ACT.
  * Pointwise transforms are single fused custom DVE instructions:
      AFFCLIP clip(a*x+b,0,1); SOLA min(x,1-x); POST8 sum(x>=k/4)*0.25.
  * Contrast mean: DVE free-dim reduce -> gpsimd partition_all_reduce ->
    gpsimd scale into the per-item coefficient column read by a paired
    AFFCLIP ("caff").
"""

import contextlib

import numpy as np

import concourse.bacc as bacc
import concourse.bass as bass
import concourse.bass_isa as bass_isa
import concourse.mybir as mybir
from concourse.bass import ds

# ---------------------------------------------------------------- constants
B, C, H, W = 128, 3, 224, 224
NSTEP = 2
NCORES = 8
BL = B // NCORES          # images per core = 16
P = 112                   # partitions (= H/2)
FPI = C * 2 * W           # free elems per image per partition = 1344
CHUNK = 2 * W             # per-channel chunk = 448 (one PSUM bank)
NSCR = 2                  # flip_w scratch slots
HALVES = 2
HB = BL // HALVES         # images per half = 8
NPIX = C * H * W

# SBUF image-slot map (offsets into one big [P, IMG_FREE] tensor)
DUMMY_V = BL * FPI             # written by DVE pad ops (wait-chained)
DUMMY_G = (BL + 1) * FPI       # read-only pad source (zeroed once)
DUMMY_A = (BL + 2) * FPI       # written by ACT pad ops (wait-chained)
SCR_OFF = [(BL + 3 + k) * FPI for k in range(NSCR)]
NSLOT_TOT = BL + 3 + NSCR
IMG_FREE = NSLOT_TOT * FPI
MAX_OFF = IMG_FREE - FPI

AFF_A = {3: 1.5, 6: -1.0}        # brightness / invert
AFF_B = {3: 0.0, 6: 1.0}
MEAN_SCALE = -0.5 / float(NPIX)  # contrast b = -0.5 * mean

FLIP_W = "flip_w"
PE_CATS = ("flip_s", "flip_sw")
ALL_CATS = ("mean", "aff", "sol", "post", FLIP_W) + PE_CATS

# ------------------------------------------------------------ custom DVE ops
_REGISTERED = {}


def _register_op(name, spec, subdim=False):
    import concourse.dve_ops as dmod
    from concourse.dve_ops import DveOp, OPS, has_src1
    from concourse.dve_spec import lower
    from concourse.dve_uop import DveOpSpec

    if name in _REGISTERED:
        return _REGISTERED[name]
    op = DveOp(name, spec, subdim, uops_sha={})
    OPS.append(op)
    dmod.CUSTOM_DVE_SPECS[name] = spec
    dmod._SUB_OPCODE_FOR_NAME[name] = dmod._CUSTOM_DVE_ROW_BASE + len(OPS) - 1
    assert dmod._SUB_OPCODE_FOR_NAME[name] < 0x20, "custom DVE row overflow"
    shas = {}
    for ver in ("v3", "v4"):
        res = DveOpSpec(
            name=name,
            opcode=dmod.get_dve_sub_opcode(name),
            uops=lower(spec, ver=ver),
            rd1_en=has_src1(spec),
        )
        shas[ver] = res.sha(ver)
    op2 = DveOp(name, spec, subdim, uops_sha=shas)
    OPS[OPS.index(op)] = op2
    _REGISTERED[name] = op2
    return op2


def _get_ops():
    from concourse.dve_spec import C0, C1, C2, One, Spec, Src0, Zero, maxx, minn

    affclip = _register_op(
        "AFFCLIP_AUG",
        Spec(
            body=maxx(minn(Src0 * C0 + C1, One), Zero),
            reference=lambda in0, in1, s0, s1, imm2: np.clip(
                in0 * s0 + s1, 0.0, 1.0
            ).astype(np.float32),
        ),
    )
    sola = _register_op(
        "SOLA_AUG",
        Spec(
            body=minn(Src0, One - Src0),
            reference=lambda in0, in1, s0, s1, imm2: np.minimum(
                in0, 1.0 - in0
            ).astype(np.float32),
        ),
    )
    post8 = _register_op(
        "POST8_AUG",
        Spec(
            body=((Src0 >= C0) + ((Src0 >= C1) + ((Src0 >= C2) + (Src0 >= One))))
            * C0,
            reference=lambda in0, in1, s0, s1, imm2: (
                (in0 >= s0).astype(np.float32)
                + (in0 >= s1)
                + (in0 >= imm2)
                + (in0 >= 1.0)
            ).astype(np.float32)
            * s0,
        ),
    )
    return affclip, sola, post8


# ------------------------------------------------------------- host planning
def plan_core(samples_core):
    """samples_core: [2, BL] -> dict[(h, r, cat)] = list of image indices."""
    plan = {}

    def add(h, r, cat, img):
        plan.setdefault((h, r, cat), []).append(img)

    for i in range(BL):
        h = i // HB
        t0, t1 = int(samples_core[0][i]), int(samples_core[1][i])
        if t0 in (1, 2) and t1 in (1, 2):
            fs = (1 if t0 == 2 else 0) ^ (1 if t1 == 2 else 0)
            fw = (1 if t0 == 1 else 0) ^ (1 if t1 == 1 else 0)
            if fs or fw:
                add(h, 0, {(1, 0): "flip_s", (0, 1): "flip_w",
                           (1, 1): "flip_sw"}[(fs, fw)], i)
            continue
        for r, t in enumerate((t0, t1)):
            if t == 0:
                continue
            cat = {1: "flip_w", 2: "flip_s", 3: "aff", 4: "mean", 5: "sol",
                   6: "aff", 7: "post"}[t]
            add(h, r, cat, i)
    return plan


class Schedule:
    """Deterministic emission order + static sem thresholds from padded (max)
    counts; min counts tell which queue tails may be pads on some core (those
    get same-engine wait chains for the shared dummy slots)."""

    def __init__(self, counts, mins):
        self.counts = counts
        self.mins = mins
        iv = ia = ig = it = 0
        self.blocks = []      # (h, r, {"v": [...], "a": [...], "g": [...], "t": [...]})
        self.cum_v, self.cum_a, self.cum_g, self.cum_t = {}, {}, {}, {}
        self.naff = sum(counts[(h, r, "aff")] for h in range(2) for r in range(2))
        self.nmean = sum(counts[(h, r, "mean")] for h in range(2) for r in range(2))
        self.n_memset = 3 + NSCR
        ig += self.n_memset
        aff_col = 0
        mean_col = 0
        fw_ord = 0
        fw_back_idx = []       # ACT op index per flip_w ordinal
        last_dummy_v = None
        last_dummy_a = None
        last_evac = {}         # chunk c -> ACT idx of last evac using psum c
        for h in range(HALVES):
            for r in range(NSTEP):
                ops = {"v": [], "a": [], "g": [], "t": []}
                nm = counts[(h, r, "mean")]
                mred_idx = []
                for k in range(nm):
                    iv += 1
                    mred_idx.append(iv)
                    ops["v"].append(dict(kind="meanred", cat="mean", k=k,
                                         col=mean_col + k))
                mul_idx = []
                for k in range(nm):
                    ig += 1
                    ops["g"].append(dict(kind="allred", k=k, col=mean_col + k,
                                         wait_v=mred_idx[k]))
                    ig += 1
                    ops["g"].append(dict(kind="meanmul", k=k, col=mean_col + k,
                                         wait_g=ig - 1))
                    mul_idx.append(ig)
                for k in range(counts[(h, r, "aff")]):
                    iv += 1
                    w = None
                    if k >= mins[(h, r, "aff")]:
                        w, last_dummy_v = last_dummy_v, iv
                    ops["v"].append(dict(kind="aff", cat="aff", k=k,
                                         col=aff_col, wait_v=w))
                    aff_col += 1
                for k in range(counts[(h, r, "sol")]):
                    iv += 1
                    w = None
                    if k >= mins[(h, r, "sol")]:
                        w, last_dummy_v = last_dummy_v, iv
                    ops["v"].append(dict(kind="sol", cat="sol", k=k, wait_v=w))
                for k in range(counts[(h, r, "post")]):
                    iv += 1
                    w = None
                    if k >= mins[(h, r, "post")]:
                        w, last_dummy_v = last_dummy_v, iv
                    ops["v"].append(dict(kind="post", cat="post", k=k, wait_v=w))
                for k in range(nm):
                    iv += 1
                    w = None
                    if k >= mins[(h, r, "mean")]:
                        w, last_dummy_v = last_dummy_v, iv
                    ops["v"].append(dict(kind="caff", cat="mean", k=k,
                                         col=mean_col + k, wait_g=mul_idx[k],
                                         wait_v=w))
                mean_col += nm
                # flip_w: gpsimd copy-out (w-reversed) -> scratch, ACT copy-back
                for k in range(counts[(h, r, FLIP_W)]):
                    ig += 1
                    g = dict(kind="cpout", cat=FLIP_W, k=k, scr=fw_ord % NSCR)
                    if fw_ord >= NSCR:
                        g["wait_a"] = fw_back_idx[fw_ord - NSCR]
                    ops["g"].append(g)
                    ia += 1
                    ops["a"].append(dict(kind="cpback", cat=FLIP_W, k=k,
                                         scr=fw_ord % NSCR, wait_g=ig))
                    fw_back_idx.append(ia)
                    fw_ord += 1
                # flip_s / flip_sw: PE partition-reversal + ACT evacuation
                for cat in PE_CATS:
                    for k in range(counts[(h, r, cat)]):
                        maybe_pad = k >= mins[(h, r, cat)]
                        for c in range(C):
                            it += 1
                            t = dict(kind="mm", cat=cat, k=k, c=c)
                            if c in last_evac:
                                t["wait_a"] = last_evac[c]
                            ops["t"].append(t)
                            ia += 1
                            a = dict(kind="evac", cat=cat, k=k, c=c, wait_t=it)
                            if maybe_pad:
                                a["wait_a"], last_dummy_a = last_dummy_a, ia
                            ops["a"].append(a)
                            last_evac[c] = ia
                self.blocks.append((h, r, ops))
                self.cum_v[(h, r)] = iv
                self.cum_a[(h, r)] = ia
                self.cum_g[(h, r)] = ig
                self.cum_t[(h, r)] = it
        self.nv, self.na, self.ng, self.nt = iv, ia, ig, it


def build_tables(plan, sched):
    """Per-core offset tables in exact emission order."""
    qv, qa, qg, qt = [], [], [], []
    for h, r, ops in sched.blocks:
        for op in ops["v"]:
            items = plan.get((h, r, op["cat"]), [])
            k = op["k"]
            if op["kind"] == "meanred":
                qv.append(items[k] * FPI if k < len(items) else DUMMY_G)
            else:
                qv.append(items[k] * FPI if k < len(items) else DUMMY_V)
        for op in ops["g"]:
            if op["kind"] in ("allred", "meanmul"):
                qg.append(0)
            else:
                items = plan.get((h, r, op["cat"]), [])
                k = op["k"]
                qg.append(items[k] * FPI if k < len(items) else DUMMY_G)
        for op in ops["a"]:
            items = plan.get((h, r, op["cat"]), [])
            k = op["k"]
            if op["kind"] == "cpback":
                qa.append(items[k] * FPI if k < len(items) else SCR_OFF[op["scr"]])
            else:  # evac
                qa.append(items[k] * FPI if k < len(items) else DUMMY_A)
        for op in ops["t"]:
            items = plan.get((h, r, op["cat"]), [])
            k = op["k"]
            qt.append(items[k] * FPI if k < len(items) else DUMMY_G)
    return qv, qa, qg, qt


def build_coeffs(plan, sched, samples_core):
    ca = np.ones(max(sched.naff, 1), dtype=np.float32)
    cb = np.zeros(max(sched.naff, 1), dtype=np.float32)
    for h, r, ops in sched.blocks:
        items = plan.get((h, r, "aff"), [])
        for op in ops["v"]:
            if op["kind"] != "aff":
                continue
            k = op["k"]
            if k < len(items):
                t = int(samples_core[r][items[k]])
                ca[op["col"]] = AFF_A[t]
                cb[op["col"]] = AFF_B[t]
    return ca, cb


# ------------------------------------------------------------- program build
def build_program(counts, mins):
    sched = Schedule(counts, mins)
    AFFCLIP, SOLA, POST8 = _get_ops()

    nqv, nqa = max(sched.nv, 1), max(sched.na, 1)
    nqg = max(sched.ng - sched.n_memset, 1)
    nqt = max(sched.nt, 1)
    naff, nmean = max(sched.naff, 1), max(sched.nmean, 1)

    nc = bacc.Bacc()
    x_t = nc.dram_tensor("x", [BL, C, H, W], mybir.dt.float32, kind="ExternalInput")
    qv_t = nc.dram_tensor("qv", [1, nqv], mybir.dt.int32, kind="ExternalInput")
    qa_t = nc.dram_tensor("qa", [1, nqa], mybir.dt.int32, kind="ExternalInput")
    qg_t = nc.dram_tensor("qg", [1, nqg], mybir.dt.int32, kind="ExternalInput")
    qt_t = nc.dram_tensor("qt", [1, nqt], mybir.dt.int32, kind="ExternalInput")
    ca_t = nc.dram_tensor("ca", [P, naff], mybir.dt.float32, kind="ExternalInput")
    cb_t = nc.dram_tensor("cb", [P, naff], mybir.dt.float32, kind="ExternalInput")
    j_t = nc.dram_tensor("jmat", [P, P], mybir.dt.float32, kind="ExternalInput")
    y_t = nc.dram_tensor("y", [BL, C, H, W], mybir.dt.float32, kind="ExternalOutput")

    img = nc.alloc_sbuf_tensor("img", [P, IMG_FREE], mybir.dt.float32).ap()
    qv_s = nc.alloc_sbuf_tensor("qv_s", [1, nqv], mybir.dt.int32).ap()
    qa_s = nc.alloc_sbuf_tensor("qa_s", [1, nqa], mybir.dt.int32).ap()
    qg_s = nc.alloc_sbuf_tensor("qg_s", [1, nqg], mybir.dt.int32).ap()
    qt_s = nc.alloc_sbuf_tensor("qt_s", [1, nqt], mybir.dt.int32).ap()
    ca_s = nc.alloc_sbuf_tensor("ca_s", [P, naff], mybir.dt.float32).ap()
    cb_s = nc.alloc_sbuf_tensor("cb_s", [P, naff], mybir.dt.float32).ap()
    j_s = nc.alloc_sbuf_tensor("j_s", [P, P], mybir.dt.float32).ap()
    cm_s = nc.alloc_sbuf_tensor("cm_s", [P, nmean], mybir.dt.float32).ap()
    red_s = nc.alloc_sbuf_tensor("red_s", [P, nmean], mybir.dt.float32).ap()
    red2_s = nc.alloc_sbuf_tensor("red2_s", [P, nmean], mybir.dt.float32).ap()
    psum = [
        nc.alloc_psum_tensor(f"ps{c}", [P, CHUNK], mybir.dt.float32).ap()
        for c in range(C)
    ]

    NTBL = 7
    tbl = nc.alloc_semaphore("tbl")
    ld = [nc.alloc_semaphore(f"ld{h}") for h in range(HALVES)]
    st = nc.alloc_semaphore("st")
    pv = nc.alloc_semaphore("pv")
    pa = nc.alloc_semaphore("pa")
    pg = nc.alloc_semaphore("pg")
    pt = nc.alloc_semaphore("pt")

    xr = x_t.rearrange("b c h w -> b h c w")
    yr = y_t.rearrange("b c h w -> b h c w")

    def img_view(i):
        return img[:, i * FPI : (i + 1) * FPI].rearrange(
            "p (c s w) -> p c s w", c=C, s=2, w=W
        )

    def dyn_flat(val):
        return img[:, ds(val, FPI)]

    def dyn_view(val):
        return img[:, ds(val, FPI)].rearrange("p (c s w) -> p c s w", c=C, s=2, w=W)

    def vload(eng, ap):
        # value_load minus the runtime-assert instruction: the error
        # notification it emits faults the device under this runtime.
        tmp = eng.alloc_register(f"vl_{nc.next_id()}")
        eng.reg_load(tmp, ap)
        val = eng.snap(tmp, donate=True)
        return nc.s_assert_within(val, 0, MAX_OFF, skip_runtime_assert=True)

    def loadq(eng, qtab, a, n):
        return [vload(eng, qtab[0:1, a + j : a + j + 1]) for j in range(n)]

    def head_waits(eng, h, r, cums):
        eng.wait_ge(ld[h], 16 * 2 * HB)
        if r == 1:
            for sem, cum in cums:
                eng.wait_ge(sem, cum[(h, 0)])

    with contextlib.ExitStack() as exit_ctx, nc.Block() as block:

        @block.sync
        def _(sync):
            for a, b_ in ((qv_s, qv_t), (qa_s, qa_t), (qg_s, qg_t), (qt_s, qt_t),
                          (ca_s, ca_t), (cb_s, cb_t), (j_s, j_t)):
                sync.dma_start(a[:, :], b_[:, :]).then_inc(tbl, 16)
            for h in range(HALVES):
                for i in range(h * HB, (h + 1) * HB):
                    v = img_view(i)
                    sync.dma_start(v[:, :, 0, :], xr[i, 0:P, :, :]).then_inc(
                        ld[h], 16
                    )
                    sync.dma_start(v[:, :, 1, :], xr[i, P:H, :, :]).then_inc(
                        ld[h], 16
                    )
            for h in range(HALVES):
                sync.wait_ge(pv, sched.cum_v[(h, 1)])
                sync.wait_ge(pa, sched.cum_a[(h, 1)])
                sync.wait_ge(pg, sched.cum_g[(h, 1)])
                for i in range(h * HB, (h + 1) * HB):
                    v = img_view(i)
                    sync.dma_start(yr[i, 0:P, :, :], v[:, :, 0, :]).then_inc(st, 16)
                    sync.dma_start(yr[i, P:H, :, :], v[:, :, 1, :]).then_inc(st, 16)
            sync.wait_ge(st, 16 * 2 * BL)

        @block.vector
        def _(vector):
            vector.wait_ge(tbl, 16 * NTBL)
            vector.wait_ge(pg, sched.n_memset)
            qi = 0
            for h, r, ops in sched.blocks:
                if not ops["v"]:
                    continue
                head_waits(vector, h, r,
                           [(pa, sched.cum_a), (pg, sched.cum_g),
                            (pv, sched.cum_v), (pt, sched.cum_t)])
                offs = loadq(vector, qv_s, qi, len(ops["v"]))
                for j, op in enumerate(ops["v"]):
                    if op.get("wait_v") is not None:
                        vector.wait_ge(pv, op["wait_v"])
                    src = dyn_flat(offs[j])
                    if op["kind"] == "meanred":
                        vector.tensor_reduce(
                            red_s[:, op["col"] : op["col"] + 1], src,
                            mybir.AxisListType.X, mybir.AluOpType.add,
                        ).then_inc(pv, 1)
                    elif op["kind"] == "caff":
                        vector.wait_ge(pg, op["wait_g"])
                        vector._custom_dve(
                            AFFCLIP, out=src, in0=src, s0=1.5,
                            s1=cm_s[:, op["col"] : op["col"] + 1],
                        ).then_inc(pv, 1)
                    elif op["kind"] == "aff":
                        vector._custom_dve(
                            AFFCLIP, out=src, in0=src,
                            s0=ca_s[:, op["col"] : op["col"] + 1],
                            s1=cb_s[:, op["col"] : op["col"] + 1],
                        ).then_inc(pv, 1)
                    elif op["kind"] == "sol":
                        vector._custom_dve(SOLA, out=src, in0=src).then_inc(pv, 1)
                    else:
                        vector._custom_dve(
                            POST8, out=src, in0=src, s0=0.25, s1=0.5, imm2=0.75
                        ).then_inc(pv, 1)
                qi += len(ops["v"])

        @block.gpsimd
        def _(gpsimd):
            for off in (DUMMY_V, DUMMY_G, DUMMY_A, *SCR_OFF):
                gpsimd.memset(img[:, off : off + FPI], 0.0).then_inc(pg, 1)
            gpsimd.wait_ge(tbl, 16 * NTBL)
            gpsimd.wait_ge(pg, sched.n_memset)
            qi = 0
            for h, r, ops in sched.blocks:
                if not ops["g"]:
                    continue
                head_waits(gpsimd, h, r,
                           [(pa, sched.cum_a), (pv, sched.cum_v),
                            (pg, sched.cum_g), (pt, sched.cum_t)])
                offs = [
                    vload(gpsimd, qg_s[0:1, qi + j : qi + j + 1])
                    if ops["g"][j]["kind"] == "cpout" else None
                    for j in range(len(ops["g"]))
                ]
                for j, op in enumerate(ops["g"]):
                    if op["kind"] == "allred":
                        gpsimd.wait_ge(pv, op["wait_v"])
                        gpsimd.partition_all_reduce(
                            red2_s[:, op["col"] : op["col"] + 1],
                            red_s[:, op["col"] : op["col"] + 1],
                            channels=P, reduce_op=bass_isa.ReduceOp.add,
                        ).then_inc(pg, 1)
                    elif op["kind"] == "meanmul":
                        gpsimd.wait_ge(pg, op["wait_g"])
                        gpsimd.tensor_scalar_mul(
                            cm_s[:, op["col"] : op["col"] + 1],
                            red2_s[:, op["col"] : op["col"] + 1],
                            MEAN_SCALE,
                        ).then_inc(pg, 1)
                    else:  # cpout (flip_w): w-reversed copy into scratch
                        if "wait_a" in op:
                            gpsimd.wait_ge(pa, op["wait_a"])
                        v = dyn_view(offs[j])
                        dst = img[
                            :, SCR_OFF[op["scr"]] : SCR_OFF[op["scr"]] + FPI
                        ].rearrange("p (c s w) -> p c s w", c=C, s=2, w=W)
                        gpsimd.tensor_copy(dst, v[:, :, :, ::-1]).then_inc(pg, 1)
                qi += len(ops["g"])

        @block.tensor
        def _(tensor):
            tensor.wait_ge(tbl, 16 * NTBL)
            tensor.wait_ge(pg, sched.n_memset)
            qi = 0
            for h, r, ops in sched.blocks:
                if not ops["t"]:
                    continue
                head_waits(tensor, h, r,
                           [(pa, sched.cum_a), (pv, sched.cum_v),
                            (pg, sched.cum_g), (pt, sched.cum_t)])
                offs = loadq(tensor, qt_s, qi, len(ops["t"]))
                for j, op in enumerate(ops["t"]):
                    if "wait_a" in op:
                        tensor.wait_ge(pa, op["wait_a"])
                    c = op["c"]
                    rhs = dyn_flat(offs[j])[:, c * CHUNK : (c + 1) * CHUNK]
                    tensor.matmul(
                        psum[c][:, :], j_s[:, :], rhs, start=True, stop=True
                    ).then_inc(pt, 1)
                qi += len(ops["t"])

        @block.scalar
        def _(scalar):
            scalar.wait_ge(tbl, 16 * NTBL)
            scalar.wait_ge(pg, sched.n_memset)
            qi = 0
            for h, r, ops in sched.blocks:
                if not ops["a"]:
                    continue
                head_waits(scalar, h, r,
                           [(pv, sched.cum_v), (pg, sched.cum_g),
                            (pa, sched.cum_a), (pt, sched.cum_t)])
                offs = loadq(scalar, qa_s, qi, len(ops["a"]))
                for j, op in enumerate(ops["a"]):
                    if op.get("wait_a") is not None:
                        scalar.wait_ge(pa, op["wait_a"])
                    if op["kind"] == "cpback":
                        scalar.wait_ge(pg, op["wait_g"])
                        s = img[:, SCR_OFF[op["scr"]] : SCR_OFF[op["scr"]] + FPI]
                        scalar.activation(
                            dyn_flat(offs[j]), s, mybir.ActivationFunctionType.Copy
                        ).then_inc(pa, 1)
                    else:  # evac: psum chunk -> home slot with s-swap (+w-rev)
                        scalar.wait_ge(pt, op["wait_t"])
                        c = op["c"]
                        pview = psum[c].rearrange("p (s w) -> p s w", s=2, w=W)
                        out = dyn_view(offs[j])[:, c]
                        out = out[:, ::-1, ::-1] if op["cat"] == "flip_sw" \
                            else out[:, ::-1, :]
                        scalar.activation(
                            out, pview, mybir.ActivationFunctionType.Copy
                        ).then_inc(pa, 1)
                qi += len(ops["a"])

    nc.compile()
    return nc, sched


# ---------------------------------------------------------------- entrypoint
def _prepare(samples):
    samples = np.asarray(samples)
    plans = [plan_core(samples[:, c * BL : (c + 1) * BL]) for c in range(NCORES)]
    counts, mins = {}, {}
    for h in range(HALVES):
        for r in range(NSTEP):
            for cat in ALL_CATS:
                ns = [len(p.get((h, r, cat), [])) for p in plans]
                counts[(h, r, cat)] = max(ns)
                mins[(h, r, cat)] = min(ns)
    return plans, counts, mins


def make_in_maps(x, samples, plans, sched):
    samples = np.asarray(samples)
    jmat = np.eye(P, dtype=np.float32)[::-1].copy()

    def tab(lst):
        return (np.asarray(lst, np.int32).reshape(1, -1)
                if lst else np.zeros((1, 1), np.int32))

    in_maps = []
    for c in range(NCORES):
        qv, qa, qg, qt = build_tables(plans[c], sched)
        ca, cb = build_coeffs(plans[c], sched, samples[:, c * BL : (c + 1) * BL])
        in_maps.append(
            dict(
                x=np.ascontiguousarray(x[c * BL : (c + 1) * BL]),
                qv=tab(qv), qa=tab(qa), qg=tab(qg), qt=tab(qt),
                ca=np.tile(ca.reshape(1, -1), (P, 1)),
                cb=np.tile(cb.reshape(1, -1), (P, 1)),
                jmat=jmat,
            )
        )
    return in_maps


def kernel(x, prob, samples):
    from concourse.bass_utils import run_bass_kernel_spmd

    x = np.ascontiguousarray(np.asarray(x, dtype=np.float32))
    plans, counts, mins = _prepare(samples)
    nc, sched = build_program(counts, mins)
    in_maps = make_in_maps(x, samples, plans, sched)
    res = run_bass_kernel_spmd(nc, in_maps, core_ids=list(range(NCORES)))
    out = np.concatenate([res.results[c]["y"] for c in range(NCORES)], axis=0)
    return out.reshape(B, C, H, W).astype(np.float32)


# revision 9
# speedup vs baseline: 1.0200x; 1.0200x over previous
"""Trainium2 Bass kernel for nn_Data_augV4 (per-image augmentation routing).

Semantics (matches the JAX reference):
  for step in range(2):
      per image b: out = TFS[samples[step][b]](out), TFS =
      [identity, flip_lr(W), flip_ud(H), brightness, contrast, solarize,
       invert, posterize]
  (`prob` does not affect the output.)

Strategy:
  * Pure data parallel: 16 images per core, 8 cores, ONE SPMD program.
  * `samples` is known when kernel() builds the program, so the host plans
    per-image work queues (per primitive kind, per half/round). Queue lengths
    are padded to the max across cores so every core runs the same program;
    per-item image offsets and coefficients are passed as data and read into
    registers (dynamic ds() access patterns).
  * SBUF layout per image: [112 partitions, (c:3, s:2, w:224)]; partition p
    holds rows h=p (s=0) and h=112+p (s=1). All DMAs use ascending strides.
    W-flip is a free-dim w reversal. H-flip = s-swap + partition reversal;
    the partition reversal runs on the tensor engine as J @ X with J the
    112x112 anti-identity (exact for 0/1 weights), and the s-swap/w-reversal
    fold into the PSUM->SBUF evacuation copy on ACT.
  * Pointwise transforms are single fused custom DVE instructions:
      AFFCLIP clip(a*x+b,0,1); SOLA min(x,1-x); POST8 sum(x>=k/4)*0.25.
  * Contrast mean: DVE free-dim reduce -> gpsimd partition_all_reduce ->
    gpsimd scale into the per-item coefficient column read by a paired
    AFFCLIP ("caff").
"""

import contextlib

import numpy as np

import concourse.bacc as bacc
import concourse.bass as bass
import concourse.bass_isa as bass_isa
import concourse.mybir as mybir
from concourse.bass import ds

# ---------------------------------------------------------------- constants
B, C, H, W = 128, 3, 224, 224
NSTEP = 2
NCORES = 8
BL = B // NCORES          # images per core = 16
P = 112                   # partitions (= H/2)
FPI = C * 2 * W           # free elems per image per partition = 1344
CHUNK = 2 * W             # per-channel chunk = 448 (one PSUM bank)
NSCR = 2                  # flip_w scratch slots
HALVES = 2
HB = BL // HALVES         # images per half = 8
NPIX = C * H * W

# SBUF image-slot map (offsets into one big [P, IMG_FREE] tensor)
DUMMY_V = BL * FPI             # written by DVE pad ops (wait-chained)
DUMMY_G = (BL + 1) * FPI       # read-only pad source (zeroed once)
DUMMY_A = (BL + 2) * FPI       # written by ACT pad ops (wait-chained)
SCR_OFF = [(BL + 3 + k) * FPI for k in range(NSCR)]
NSLOT_TOT = BL + 3 + NSCR
IMG_FREE = NSLOT_TOT * FPI
MAX_OFF = IMG_FREE - FPI

AFF_A = {3: 1.5, 6: -1.0}        # brightness / invert
AFF_B = {3: 0.0, 6: 1.0}
MEAN_SCALE = -0.5 / float(NPIX)  # contrast b = -0.5 * mean

FLIP_W = "flip_w"
PE_CATS = ("flip_s", "flip_sw")
ALL_CATS = ("mean", "aff", "sol", "post", FLIP_W) + PE_CATS

# ------------------------------------------------------------ custom DVE ops
_REGISTERED = {}


def _register_op(name, spec, subdim=False):
    import concourse.dve_ops as dmod
    from concourse.dve_ops import DveOp, OPS, has_src1
    from concourse.dve_spec import lower
    from concourse.dve_uop import DveOpSpec

    if name in _REGISTERED:
        return _REGISTERED[name]
    op = DveOp(name, spec, subdim, uops_sha={})
    OPS.append(op)
    dmod.CUSTOM_DVE_SPECS[name] = spec
    dmod._SUB_OPCODE_FOR_NAME[name] = dmod._CUSTOM_DVE_ROW_BASE + len(OPS) - 1
    assert dmod._SUB_OPCODE_FOR_NAME[name] < 0x20, "custom DVE row overflow"
    shas = {}
    for ver in ("v3", "v4"):
        res = DveOpSpec(
            name=name,
            opcode=dmod.get_dve_sub_opcode(name),
            uops=lower(spec, ver=ver),
            rd1_en=has_src1(spec),
        )
        shas[ver] = res.sha(ver)
    op2 = DveOp(name, spec, subdim, uops_sha=shas)
    OPS[OPS.index(op)] = op2
    _REGISTERED[name] = op2
    return op2


def _get_ops():
    from concourse.dve_spec import C0, C1, C2, One, Spec, Src0, Zero, maxx, minn

    affclip = _register_op(
        "AFFCLIP_AUG",
        Spec(
            body=maxx(minn(Src0 * C0 + C1, One), Zero),
            reference=lambda in0, in1, s0, s1, imm2: np.clip(
                in0 * s0 + s1, 0.0, 1.0
            ).astype(np.float32),
        ),
    )
    sola = _register_op(
        "SOLA_AUG",
        Spec(
            body=minn(Src0, One - Src0),
            reference=lambda in0, in1, s0, s1, imm2: np.minimum(
                in0, 1.0 - in0
            ).astype(np.float32),
        ),
    )
    post8 = _register_op(
        "POST8_AUG",
        Spec(
            body=((Src0 >= C0) + ((Src0 >= C1) + ((Src0 >= C2) + (Src0 >= One))))
            * C0,
            reference=lambda in0, in1, s0, s1, imm2: (
                (in0 >= s0).astype(np.float32)
                + (in0 >= s1)
                + (in0 >= imm2)
                + (in0 >= 1.0)
            ).astype(np.float32)
            * s0,
        ),
    )
    return affclip, sola, post8


# ------------------------------------------------------------- host planning
def plan_core(samples_core):
    """samples_core: [2, BL] -> dict[(h, r, cat)] = list of image indices."""
    plan = {}

    def add(h, r, cat, img):
        plan.setdefault((h, r, cat), []).append(img)

    for i in range(BL):
        h = i // HB
        t0, t1 = int(samples_core[0][i]), int(samples_core[1][i])
        if t0 in (1, 2) and t1 in (1, 2):
            fs = (1 if t0 == 2 else 0) ^ (1 if t1 == 2 else 0)
            fw = (1 if t0 == 1 else 0) ^ (1 if t1 == 1 else 0)
            if fs or fw:
                add(h, 0, {(1, 0): "flip_s", (0, 1): "flip_w",
                           (1, 1): "flip_sw"}[(fs, fw)], i)
            continue
        for r, t in enumerate((t0, t1)):
            if t == 0:
                continue
            cat = {1: "flip_w", 2: "flip_s", 3: "aff", 4: "mean", 5: "sol",
                   6: "aff", 7: "post"}[t]
            add(h, r, cat, i)
    return plan


class Schedule:
    """Deterministic emission order + static sem thresholds from padded (max)
    counts; min counts tell which queue tails may be pads on some core (those
    get same-engine wait chains for the shared dummy slots)."""

    def __init__(self, counts, mins):
        self.counts = counts
        self.mins = mins
        iv = ia = ig = it = 0
        self.blocks = []      # (h, r, {"v": [...], "a": [...], "g": [...], "t": [...]})
        self.cum_v, self.cum_a, self.cum_g, self.cum_t = {}, {}, {}, {}
        self.naff = sum(counts[(h, r, "aff")] for h in range(2) for r in range(2))
        self.nmean = sum(counts[(h, r, "mean")] for h in range(2) for r in range(2))
        self.n_memset = 3 + NSCR
        ig += self.n_memset
        aff_col = 0
        mean_col = 0
        fw_ord = 0
        fw_back_idx = []       # ACT op index per flip_w ordinal
        last_dummy_v = None
        last_dummy_a = None
        last_evac = {}         # chunk c -> ACT idx of last evac using psum c
        for h in range(HALVES):
            for r in range(NSTEP):
                ops = {"v": [], "a": [], "g": [], "t": []}
                nm = counts[(h, r, "mean")]
                mred_idx = []
                for k in range(nm):
                    iv += 1
                    mred_idx.append(iv)
                    ops["v"].append(dict(kind="meanred", cat="mean", k=k,
                                         col=mean_col + k))
                mul_idx = []
                for k in range(nm):
                    ig += 1
                    ops["g"].append(dict(kind="allred", k=k, col=mean_col + k,
                                         wait_v=mred_idx[k]))
                    ig += 1
                    ops["g"].append(dict(kind="meanmul", k=k, col=mean_col + k,
                                         wait_g=ig - 1))
                    mul_idx.append(ig)
                for k in range(counts[(h, r, "aff")]):
                    iv += 1
                    w = None
                    if k >= mins[(h, r, "aff")]:
                        w, last_dummy_v = last_dummy_v, iv
                    ops["v"].append(dict(kind="aff", cat="aff", k=k,
                                         col=aff_col, wait_v=w))
                    aff_col += 1
                for k in range(counts[(h, r, "sol")]):
                    iv += 1
                    w = None
                    if k >= mins[(h, r, "sol")]:
                        w, last_dummy_v = last_dummy_v, iv
                    ops["v"].append(dict(kind="sol", cat="sol", k=k, wait_v=w))
                for k in range(counts[(h, r, "post")]):
                    iv += 1
                    w = None
                    if k >= mins[(h, r, "post")]:
                        w, last_dummy_v = last_dummy_v, iv
                    ops["v"].append(dict(kind="post", cat="post", k=k, wait_v=w))
                for k in range(nm):
                    iv += 1
                    w = None
                    if k >= mins[(h, r, "mean")]:
                        w, last_dummy_v = last_dummy_v, iv
                    ops["v"].append(dict(kind="caff", cat="mean", k=k,
                                         col=mean_col + k, wait_g=mul_idx[k],
                                         wait_v=w))
                mean_col += nm
                # flip_w: gpsimd copy-out (w-reversed) -> scratch, ACT copy-back
                for k in range(counts[(h, r, FLIP_W)]):
                    ig += 1
                    g = dict(kind="cpout", cat=FLIP_W, k=k, scr=fw_ord % NSCR)
                    if fw_ord >= NSCR:
                        g["wait_a"] = fw_back_idx[fw_ord - NSCR]
                    ops["g"].append(g)
                    ia += 1
                    ops["a"].append(dict(kind="cpback", cat=FLIP_W, k=k,
                                         scr=fw_ord % NSCR, wait_g=ig))
                    fw_back_idx.append(ia)
                    fw_ord += 1
                # flip_s / flip_sw: PE partition-reversal + ACT evacuation
                for cat in PE_CATS:
                    for k in range(counts[(h, r, cat)]):
                        maybe_pad = k >= mins[(h, r, cat)]
                        for c in range(C):
                            it += 1
                            t = dict(kind="mm", cat=cat, k=k, c=c)
                            if c in last_evac:
                                t["wait_a"] = last_evac[c]
                            ops["t"].append(t)
                            ia += 1
                            a = dict(kind="evac", cat=cat, k=k, c=c, wait_t=it)
                            if maybe_pad:
                                a["wait_a"], last_dummy_a = last_dummy_a, ia
                            ops["a"].append(a)
                            last_evac[c] = ia
                self.blocks.append((h, r, ops))
                self.cum_v[(h, r)] = iv
                self.cum_a[(h, r)] = ia
                self.cum_g[(h, r)] = ig
                self.cum_t[(h, r)] = it
        self.nv, self.na, self.ng, self.nt = iv, ia, ig, it


def build_tables(plan, sched):
    """Per-core offset tables in exact emission order."""
    qv, qa, qg, qt = [], [], [], []
    for h, r, ops in sched.blocks:
        for op in ops["v"]:
            items = plan.get((h, r, op["cat"]), [])
            k = op["k"]
            if op["kind"] == "meanred":
                qv.append(items[k] * FPI if k < len(items) else DUMMY_G)
            else:
                qv.append(items[k] * FPI if k < len(items) else DUMMY_V)
        for op in ops["g"]:
            if op["kind"] in ("allred", "meanmul"):
                qg.append(0)
            else:
                items = plan.get((h, r, op["cat"]), [])
                k = op["k"]
                qg.append(items[k] * FPI if k < len(items) else DUMMY_G)
        for op in ops["a"]:
            items = plan.get((h, r, op["cat"]), [])
            k = op["k"]
            if op["kind"] == "cpback":
                qa.append(items[k] * FPI if k < len(items) else SCR_OFF[op["scr"]])
            else:  # evac
                qa.append(items[k] * FPI if k < len(items) else DUMMY_A)
        for op in ops["t"]:
            items = plan.get((h, r, op["cat"]), [])
            k = op["k"]
            qt.append(items[k] * FPI if k < len(items) else DUMMY_G)
    return qv, qa, qg, qt


def build_coeffs(plan, sched, samples_core):
    ca = np.ones(max(sched.naff, 1), dtype=np.float32)
    cb = np.zeros(max(sched.naff, 1), dtype=np.float32)
    for h, r, ops in sched.blocks:
        items = plan.get((h, r, "aff"), [])
        for op in ops["v"]:
            if op["kind"] != "aff":
                continue
            k = op["k"]
            if k < len(items):
                t = int(samples_core[r][items[k]])
                ca[op["col"]] = AFF_A[t]
                cb[op["col"]] = AFF_B[t]
    return ca, cb


# ------------------------------------------------------------- program build
def build_program(counts, mins):
    sched = Schedule(counts, mins)
    AFFCLIP, SOLA, POST8 = _get_ops()

    nqv, nqa = max(sched.nv, 1), max(sched.na, 1)
    nqg = max(sched.ng - sched.n_memset, 1)
    nqt = max(sched.nt, 1)
    naff, nmean = max(sched.naff, 1), max(sched.nmean, 1)

    nc = bacc.Bacc()
    x_t = nc.dram_tensor("x", [BL, C, H, W], mybir.dt.float32, kind="ExternalInput")
    qv_t = nc.dram_tensor("qv", [1, nqv], mybir.dt.int32, kind="ExternalInput")
    qa_t = nc.dram_tensor("qa", [1, nqa], mybir.dt.int32, kind="ExternalInput")
    qg_t = nc.dram_tensor("qg", [1, nqg], mybir.dt.int32, kind="ExternalInput")
    qt_t = nc.dram_tensor("qt", [1, nqt], mybir.dt.int32, kind="ExternalInput")
    ca_t = nc.dram_tensor("ca", [P, naff], mybir.dt.float32, kind="ExternalInput")
    cb_t = nc.dram_tensor("cb", [P, naff], mybir.dt.float32, kind="ExternalInput")
    j_t = nc.dram_tensor("jmat", [P, P], mybir.dt.float32, kind="ExternalInput")
    y_t = nc.dram_tensor("y", [BL, C, H, W], mybir.dt.float32, kind="ExternalOutput")

    img = nc.alloc_sbuf_tensor("img", [P, IMG_FREE], mybir.dt.float32).ap()
    qv_s = nc.alloc_sbuf_tensor("qv_s", [1, nqv], mybir.dt.int32).ap()
    qa_s = nc.alloc_sbuf_tensor("qa_s", [1, nqa], mybir.dt.int32).ap()
    qg_s = nc.alloc_sbuf_tensor("qg_s", [1, nqg], mybir.dt.int32).ap()
    qt_s = nc.alloc_sbuf_tensor("qt_s", [1, nqt], mybir.dt.int32).ap()
    ca_s = nc.alloc_sbuf_tensor("ca_s", [P, naff], mybir.dt.float32).ap()
    cb_s = nc.alloc_sbuf_tensor("cb_s", [P, naff], mybir.dt.float32).ap()
    j_s = nc.alloc_sbuf_tensor("j_s", [P, P], mybir.dt.float32).ap()
    cm_s = nc.alloc_sbuf_tensor("cm_s", [P, nmean], mybir.dt.float32).ap()
    red_s = nc.alloc_sbuf_tensor("red_s", [P, nmean], mybir.dt.float32).ap()
    red2_s = nc.alloc_sbuf_tensor("red2_s", [P, nmean], mybir.dt.float32).ap()
    psum = [
        nc.alloc_psum_tensor(f"ps{c}", [P, CHUNK], mybir.dt.float32).ap()
        for c in range(C)
    ]

    NTBL = 7
    tbl = nc.alloc_semaphore("tbl")
    ld = [nc.alloc_semaphore(f"ld{h}") for h in range(HALVES)]
    st = nc.alloc_semaphore("st")
    pv = nc.alloc_semaphore("pv")
    pa = nc.alloc_semaphore("pa")
    pg = nc.alloc_semaphore("pg")
    pt = nc.alloc_semaphore("pt")

    xr = x_t.rearrange("b c h w -> b h c w")
    yr = y_t.rearrange("b c h w -> b h c w")

    def img_view(i):
        return img[:, i * FPI : (i + 1) * FPI].rearrange(
            "p (c s w) -> p c s w", c=C, s=2, w=W
        )

    def dyn_flat(val):
        return img[:, ds(val, FPI)]

    def dyn_view(val):
        return img[:, ds(val, FPI)].rearrange("p (c s w) -> p c s w", c=C, s=2, w=W)

    def vload(eng, ap):
        # value_load minus the runtime-assert instruction: the error
        # notification it emits faults the device under this runtime.
        tmp = eng.alloc_register(f"vl_{nc.next_id()}")
        eng.reg_load(tmp, ap)
        val = eng.snap(tmp, donate=True)
        return nc.s_assert_within(val, 0, MAX_OFF, skip_runtime_assert=True)

    def loadq(eng, qtab, a, n):
        # one batched multi-register load per block instead of n serial loads
        if n == 0:
            return []
        regs = [eng.alloc_register(f"q_{nc.next_id()}") for _ in range(n)]
        eng.reg_load(regs, qtab[0:1, a : a + n])
        return [
            nc.s_assert_within(eng.snap(r, donate=True), 0, MAX_OFF,
                               skip_runtime_assert=True)
            for r in regs
        ]

    def head_waits(eng, h, r, cums):
        eng.wait_ge(ld[h], 16 * 2 * HB)
        if r == 1:
            for sem, cum in cums:
                eng.wait_ge(sem, cum[(h, 0)])

    with contextlib.ExitStack() as exit_ctx, nc.Block() as block:

        @block.sync
        def _(sync):
            for a, b_ in ((qv_s, qv_t), (qa_s, qa_t), (qg_s, qg_t), (qt_s, qt_t),
                          (ca_s, ca_t), (cb_s, cb_t), (j_s, j_t)):
                sync.dma_start(a[:, :], b_[:, :]).then_inc(tbl, 16)
            for h in range(HALVES):
                for i in range(h * HB, (h + 1) * HB):
                    v = img_view(i)
                    sync.dma_start(v[:, :, 0, :], xr[i, 0:P, :, :]).then_inc(
                        ld[h], 16
                    )
                    sync.dma_start(v[:, :, 1, :], xr[i, P:H, :, :]).then_inc(
                        ld[h], 16
                    )
            for h in range(HALVES):
                sync.wait_ge(pv, sched.cum_v[(h, 1)])
                sync.wait_ge(pa, sched.cum_a[(h, 1)])
                sync.wait_ge(pg, sched.cum_g[(h, 1)])
                for i in range(h * HB, (h + 1) * HB):
                    v = img_view(i)
                    sync.dma_start(yr[i, 0:P, :, :], v[:, :, 0, :]).then_inc(st, 16)
                    sync.dma_start(yr[i, P:H, :, :], v[:, :, 1, :]).then_inc(st, 16)
            sync.wait_ge(st, 16 * 2 * BL)

        @block.vector
        def _(vector):
            vector.wait_ge(tbl, 16 * NTBL)
            vector.wait_ge(pg, sched.n_memset)
            qi = 0
            for h, r, ops in sched.blocks:
                if not ops["v"]:
                    continue
                head_waits(vector, h, r,
                           [(pa, sched.cum_a), (pg, sched.cum_g),
                            (pv, sched.cum_v), (pt, sched.cum_t)])
                offs = loadq(vector, qv_s, qi, len(ops["v"]))
                for j, op in enumerate(ops["v"]):
                    if op.get("wait_v") is not None:
                        vector.wait_ge(pv, op["wait_v"])
                    src = dyn_flat(offs[j])
                    if op["kind"] == "meanred":
                        vector.tensor_reduce(
                            red_s[:, op["col"] : op["col"] + 1], src,
                            mybir.AxisListType.X, mybir.AluOpType.add,
                        ).then_inc(pv, 1)
                    elif op["kind"] == "caff":
                        vector.wait_ge(pg, op["wait_g"])
                        vector._custom_dve(
                            AFFCLIP, out=src, in0=src, s0=1.5,
                            s1=cm_s[:, op["col"] : op["col"] + 1],
                        ).then_inc(pv, 1)
                    elif op["kind"] == "aff":
                        vector._custom_dve(
                            AFFCLIP, out=src, in0=src,
                            s0=ca_s[:, op["col"] : op["col"] + 1],
                            s1=cb_s[:, op["col"] : op["col"] + 1],
                        ).then_inc(pv, 1)
                    elif op["kind"] == "sol":
                        vector._custom_dve(SOLA, out=src, in0=src).then_inc(pv, 1)
                    else:
                        vector._custom_dve(
                            POST8, out=src, in0=src, s0=0.25, s1=0.5, imm2=0.75
                        ).then_inc(pv, 1)
                qi += len(ops["v"])

        @block.gpsimd
        def _(gpsimd):
            for off in (DUMMY_V, DUMMY_G, DUMMY_A, *SCR_OFF):
                gpsimd.memset(img[:, off : off + FPI], 0.0).then_inc(pg, 1)
            gpsimd.wait_ge(tbl, 16 * NTBL)
            gpsimd.wait_ge(pg, sched.n_memset)
            qi = 0
            for h, r, ops in sched.blocks:
                if not ops["g"]:
                    continue
                head_waits(gpsimd, h, r,
                           [(pa, sched.cum_a), (pv, sched.cum_v),
                            (pg, sched.cum_g), (pt, sched.cum_t)])
                offs = loadq(gpsimd, qg_s, qi, len(ops["g"]))
                for j, op in enumerate(ops["g"]):
                    if op["kind"] == "allred":
                        gpsimd.wait_ge(pv, op["wait_v"])
                        gpsimd.partition_all_reduce(
                            red2_s[:, op["col"] : op["col"] + 1],
                            red_s[:, op["col"] : op["col"] + 1],
                            channels=P, reduce_op=bass_isa.ReduceOp.add,
                        ).then_inc(pg, 1)
                    elif op["kind"] == "meanmul":
                        gpsimd.wait_ge(pg, op["wait_g"])
                        gpsimd.tensor_scalar_mul(
                            cm_s[:, op["col"] : op["col"] + 1],
                            red2_s[:, op["col"] : op["col"] + 1],
                            MEAN_SCALE,
                        ).then_inc(pg, 1)
                    else:  # cpout (flip_w): w-reversed copy into scratch
                        if "wait_a" in op:
                            gpsimd.wait_ge(pa, op["wait_a"])
                        v = dyn_view(offs[j])
                        dst = img[
                            :, SCR_OFF[op["scr"]] : SCR_OFF[op["scr"]] + FPI
                        ].rearrange("p (c s w) -> p c s w", c=C, s=2, w=W)
                        gpsimd.tensor_copy(dst, v[:, :, :, ::-1]).then_inc(pg, 1)
                qi += len(ops["g"])

        @block.tensor
        def _(tensor):
            tensor.wait_ge(tbl, 16 * NTBL)
            tensor.wait_ge(pg, sched.n_memset)
            qi = 0
            for h, r, ops in sched.blocks:
                if not ops["t"]:
                    continue
                head_waits(tensor, h, r,
                           [(pa, sched.cum_a), (pv, sched.cum_v),
                            (pg, sched.cum_g), (pt, sched.cum_t)])
                offs = loadq(tensor, qt_s, qi, len(ops["t"]))
                for j, op in enumerate(ops["t"]):
                    if "wait_a" in op:
                        tensor.wait_ge(pa, op["wait_a"])
                    c = op["c"]
                    rhs = dyn_flat(offs[j])[:, c * CHUNK : (c + 1) * CHUNK]
                    tensor.matmul(
                        psum[c][:, :], j_s[:, :], rhs, start=True, stop=True
                    ).then_inc(pt, 1)
                qi += len(ops["t"])

        @block.scalar
        def _(scalar):
            scalar.wait_ge(tbl, 16 * NTBL)
            scalar.wait_ge(pg, sched.n_memset)
            qi = 0
            for h, r, ops in sched.blocks:
                if not ops["a"]:
                    continue
                head_waits(scalar, h, r,
                           [(pv, sched.cum_v), (pg, sched.cum_g),
                            (pa, sched.cum_a), (pt, sched.cum_t)])
                offs = loadq(scalar, qa_s, qi, len(ops["a"]))
                for j, op in enumerate(ops["a"]):
                    if op.get("wait_a") is not None:
                        scalar.wait_ge(pa, op["wait_a"])
                    if op["kind"] == "cpback":
                        scalar.wait_ge(pg, op["wait_g"])
                        s = img[:, SCR_OFF[op["scr"]] : SCR_OFF[op["scr"]] + FPI]
                        scalar.activation(
                            dyn_flat(offs[j]), s, mybir.ActivationFunctionType.Copy
                        ).then_inc(pa, 1)
                    else:  # evac: psum chunk -> home slot with s-swap (+w-rev)
                        scalar.wait_ge(pt, op["wait_t"])
                        c = op["c"]
                        pview = psum[c].rearrange("p (s w) -> p s w", s=2, w=W)
                        out = dyn_view(offs[j])[:, c]
                        out = out[:, ::-1, ::-1] if op["cat"] == "flip_sw" \
                            else out[:, ::-1, :]
                        scalar.activation(
                            out, pview, mybir.ActivationFunctionType.Copy
                        ).then_inc(pa, 1)
                qi += len(ops["a"])

    nc.compile()
    return nc, sched


# ---------------------------------------------------------------- entrypoint
def _prepare(samples):
    samples = np.asarray(samples)
    plans = [plan_core(samples[:, c * BL : (c + 1) * BL]) for c in range(NCORES)]
    counts, mins = {}, {}
    for h in range(HALVES):
        for r in range(NSTEP):
            for cat in ALL_CATS:
                ns = [len(p.get((h, r, cat), [])) for p in plans]
                counts[(h, r, cat)] = max(ns)
                mins[(h, r, cat)] = min(ns)
    return plans, counts, mins


def make_in_maps(x, samples, plans, sched):
    samples = np.asarray(samples)
    jmat = np.eye(P, dtype=np.float32)[::-1].copy()

    def tab(lst):
        return (np.asarray(lst, np.int32).reshape(1, -1)
                if lst else np.zeros((1, 1), np.int32))

    in_maps = []
    for c in range(NCORES):
        qv, qa, qg, qt = build_tables(plans[c], sched)
        ca, cb = build_coeffs(plans[c], sched, samples[:, c * BL : (c + 1) * BL])
        in_maps.append(
            dict(
                x=np.ascontiguousarray(x[c * BL : (c + 1) * BL]),
                qv=tab(qv), qa=tab(qa), qg=tab(qg), qt=tab(qt),
                ca=np.tile(ca.reshape(1, -1), (P, 1)),
                cb=np.tile(cb.reshape(1, -1), (P, 1)),
                jmat=jmat,
            )
        )
    return in_maps


def kernel(x, prob, samples):
    from concourse.bass_utils import run_bass_kernel_spmd

    x = np.ascontiguousarray(np.asarray(x, dtype=np.float32))
    plans, counts, mins = _prepare(samples)
    nc, sched = build_program(counts, mins)
    in_maps = make_in_maps(x, samples, plans, sched)
    res = run_bass_kernel_spmd(nc, in_maps, core_ids=list(range(NCORES)))
    out = np.concatenate([res.results[c]["y"] for c in range(NCORES)], axis=0)
    return out.reshape(B, C, H, W).astype(np.float32)


# revision 10
# speedup vs baseline: 1.1283x; 1.1062x over previous
"""Trainium2 Bass kernel for nn_Data_augV4 (per-image augmentation routing).

Semantics (matches the JAX reference):
  for step in range(2):
      per image b: out = TFS[samples[step][b]](out), TFS =
      [identity, flip_lr(W), flip_ud(H), brightness, contrast, solarize,
       invert, posterize]
  (`prob` does not affect the output.)

Strategy:
  * Pure data parallel: 16 images per core, 8 cores, ONE SPMD program.
  * `samples` is known when kernel() builds the program, so the host plans
    per-image work queues (per primitive kind, per half/round). Queue lengths
    are padded to the max across cores so every core runs the same program;
    per-item image offsets and coefficients are passed as data and read into
    registers (dynamic ds() access patterns).
  * SBUF layout per image: [112 partitions, (c:3, s:2, w:224)]; partition p
    holds rows h=p (s=0) and h=112+p (s=1). All DMAs use ascending strides.
    W-flip is a free-dim w reversal. H-flip = s-swap + partition reversal;
    the partition reversal runs on the tensor engine as J @ X with J the
    112x112 anti-identity (exact for 0/1 weights), and the s-swap/w-reversal
    fold into the PSUM->SBUF evacuation copy on ACT.
  * Pointwise transforms are single fused custom DVE instructions:
      AFFCLIP clip(a*x+b,0,1); SOLA min(x,1-x); POST8 sum(x>=k/4)*0.25.
  * Contrast mean: DVE free-dim reduce -> gpsimd partition_all_reduce ->
    gpsimd scale into the per-item coefficient column read by a paired
    AFFCLIP ("caff").
"""

import contextlib

import numpy as np

import concourse.bacc as bacc
import concourse.bass as bass
import concourse.bass_isa as bass_isa
import concourse.mybir as mybir
from concourse.bass import ds

# ---------------------------------------------------------------- constants
B, C, H, W = 128, 3, 224, 224
NSTEP = 2
NCORES = 8
BL = B // NCORES          # images per core = 16
P = 112                   # partitions (= H/2)
FPI = C * 2 * W           # free elems per image per partition = 1344
CHUNK = 2 * W             # per-channel chunk = 448 (one PSUM bank)
NSCR = 2                  # flip_w scratch slots
HALVES = 2
HB = BL // HALVES         # images per half = 8
NPIX = C * H * W

# SBUF image-slot map (offsets into one big [P, IMG_FREE] tensor)
DUMMY_V = BL * FPI             # written by DVE pad ops (wait-chained)
DUMMY_G = (BL + 1) * FPI       # read-only pad source (zeroed once)
DUMMY_A = (BL + 2) * FPI       # written by ACT pad ops (wait-chained)
SCR_OFF = [(BL + 3 + k) * FPI for k in range(NSCR)]
NSLOT_TOT = BL + 3 + NSCR
IMG_FREE = NSLOT_TOT * FPI
MAX_OFF = IMG_FREE - FPI

AFF_A = {3: 1.5, 6: -1.0}        # brightness / invert
AFF_B = {3: 0.0, 6: 1.0}
MEAN_SCALE = -0.5 / float(NPIX)  # contrast b = -0.5 * mean

FLIP_W = "flip_w"
PE_CATS = ("flip_s", "flip_sw")
ALL_CATS = ("mean", "aff", "sol", "post", FLIP_W) + PE_CATS

# ------------------------------------------------------------ custom DVE ops
_REGISTERED = {}


def _register_op(name, spec, subdim=False):
    import concourse.dve_ops as dmod
    from concourse.dve_ops import DveOp, OPS, has_src1
    from concourse.dve_spec import lower
    from concourse.dve_uop import DveOpSpec

    if name in _REGISTERED:
        return _REGISTERED[name]
    op = DveOp(name, spec, subdim, uops_sha={})
    OPS.append(op)
    dmod.CUSTOM_DVE_SPECS[name] = spec
    dmod._SUB_OPCODE_FOR_NAME[name] = dmod._CUSTOM_DVE_ROW_BASE + len(OPS) - 1
    assert dmod._SUB_OPCODE_FOR_NAME[name] < 0x20, "custom DVE row overflow"
    shas = {}
    for ver in ("v3", "v4"):
        res = DveOpSpec(
            name=name,
            opcode=dmod.get_dve_sub_opcode(name),
            uops=lower(spec, ver=ver),
            rd1_en=has_src1(spec),
        )
        shas[ver] = res.sha(ver)
    op2 = DveOp(name, spec, subdim, uops_sha=shas)
    OPS[OPS.index(op)] = op2
    _REGISTERED[name] = op2
    return op2


def _get_ops():
    from concourse.dve_spec import C0, C1, C2, One, Spec, Src0, Zero, maxx, minn

    affclip = _register_op(
        "AFFCLIP_AUG",
        Spec(
            body=maxx(minn(Src0 * C0 + C1, One), Zero),
            reference=lambda in0, in1, s0, s1, imm2: np.clip(
                in0 * s0 + s1, 0.0, 1.0
            ).astype(np.float32),
        ),
    )
    sola = _register_op(
        "SOLA_AUG",
        Spec(
            body=minn(Src0, One - Src0),
            reference=lambda in0, in1, s0, s1, imm2: np.minimum(
                in0, 1.0 - in0
            ).astype(np.float32),
        ),
    )
    post8 = _register_op(
        "POST8_AUG",
        Spec(
            body=((Src0 >= C0) + ((Src0 >= C1) + ((Src0 >= C2) + (Src0 >= One))))
            * C0,
            reference=lambda in0, in1, s0, s1, imm2: (
                (in0 >= s0).astype(np.float32)
                + (in0 >= s1)
                + (in0 >= imm2)
                + (in0 >= 1.0)
            ).astype(np.float32)
            * s0,
        ),
    )
    return affclip, sola, post8


# ------------------------------------------------------------- host planning
def plan_core(samples_core):
    """samples_core: [2, BL] -> dict[(h, r, cat)] = list of image indices."""
    plan = {}

    def add(h, r, cat, img):
        plan.setdefault((h, r, cat), []).append(img)

    for i in range(BL):
        h = i // HB
        t0, t1 = int(samples_core[0][i]), int(samples_core[1][i])
        if t0 in (1, 2) and t1 in (1, 2):
            fs = (1 if t0 == 2 else 0) ^ (1 if t1 == 2 else 0)
            fw = (1 if t0 == 1 else 0) ^ (1 if t1 == 1 else 0)
            if fs or fw:
                add(h, 0, {(1, 0): "flip_s", (0, 1): "flip_w",
                           (1, 1): "flip_sw"}[(fs, fw)], i)
            continue
        for r, t in enumerate((t0, t1)):
            if t == 0:
                continue
            cat = {1: "flip_w", 2: "flip_s", 3: "aff", 4: "mean", 5: "sol",
                   6: "aff", 7: "post"}[t]
            add(h, r, cat, i)
    return plan


class Schedule:
    """Deterministic emission order + static sem thresholds from padded (max)
    counts; min counts tell which queue tails may be pads on some core (those
    get same-engine wait chains for the shared dummy slots)."""

    def __init__(self, counts, mins):
        self.counts = counts
        self.mins = mins
        iv = ia = ig = it = 0
        self.blocks = []      # (h, r, {"v": [...], "a": [...], "g": [...], "t": [...]})
        self.cum_v, self.cum_a, self.cum_g, self.cum_t = {}, {}, {}, {}
        self.naff = sum(counts[(h, r, "aff")] for h in range(2) for r in range(2))
        self.nmean = sum(counts[(h, r, "mean")] for h in range(2) for r in range(2))
        self.n_memset = 3 + NSCR
        ig += self.n_memset
        aff_col = 0
        mean_col = 0
        fw_ord = 0
        fw_back_idx = []       # ACT op index per flip_w ordinal
        last_dummy_v = None
        last_dummy_a = None
        last_evac = {}         # chunk c -> ACT idx of last evac using psum c
        # r-major order: engines run both halves of round 0 back-to-back and
        # only synchronize with other engines at the round boundary per half.
        for r in range(NSTEP):
            for h in range(HALVES):
                ops = {"v": [], "a": [], "g": [], "t": []}
                nm = counts[(h, r, "mean")]
                mred_idx = []
                for k in range(nm):
                    iv += 1
                    mred_idx.append(iv)
                    ops["v"].append(dict(kind="meanred", cat="mean", k=k,
                                         col=mean_col + k))
                mul_idx = []
                for k in range(nm):
                    ig += 1
                    ops["g"].append(dict(kind="allred", k=k, col=mean_col + k,
                                         wait_v=mred_idx[k]))
                    ig += 1
                    ops["g"].append(dict(kind="meanmul", k=k, col=mean_col + k,
                                         wait_g=ig - 1))
                    mul_idx.append(ig)
                for k in range(counts[(h, r, "aff")]):
                    iv += 1
                    w = None
                    if k >= mins[(h, r, "aff")]:
                        w, last_dummy_v = last_dummy_v, iv
                    ops["v"].append(dict(kind="aff", cat="aff", k=k,
                                         col=aff_col, wait_v=w))
                    aff_col += 1
                for k in range(counts[(h, r, "sol")]):
                    iv += 1
                    w = None
                    if k >= mins[(h, r, "sol")]:
                        w, last_dummy_v = last_dummy_v, iv
                    ops["v"].append(dict(kind="sol", cat="sol", k=k, wait_v=w))
                for k in range(counts[(h, r, "post")]):
                    iv += 1
                    w = None
                    if k >= mins[(h, r, "post")]:
                        w, last_dummy_v = last_dummy_v, iv
                    ops["v"].append(dict(kind="post", cat="post", k=k, wait_v=w))
                for k in range(nm):
                    iv += 1
                    w = None
                    if k >= mins[(h, r, "mean")]:
                        w, last_dummy_v = last_dummy_v, iv
                    ops["v"].append(dict(kind="caff", cat="mean", k=k,
                                         col=mean_col + k, wait_g=mul_idx[k],
                                         wait_v=w))
                mean_col += nm
                # flip_w: gpsimd copy-out (w-reversed) -> scratch, ACT copy-back
                for k in range(counts[(h, r, FLIP_W)]):
                    ig += 1
                    g = dict(kind="cpout", cat=FLIP_W, k=k, scr=fw_ord % NSCR)
                    if fw_ord >= NSCR:
                        g["wait_a"] = fw_back_idx[fw_ord - NSCR]
                    ops["g"].append(g)
                    ia += 1
                    ops["a"].append(dict(kind="cpback", cat=FLIP_W, k=k,
                                         scr=fw_ord % NSCR, wait_g=ig))
                    fw_back_idx.append(ia)
                    fw_ord += 1
                # flip_s / flip_sw: PE partition-reversal + ACT evacuation
                for cat in PE_CATS:
                    for k in range(counts[(h, r, cat)]):
                        maybe_pad = k >= mins[(h, r, cat)]
                        for c in range(C):
                            it += 1
                            t = dict(kind="mm", cat=cat, k=k, c=c)
                            if c in last_evac:
                                t["wait_a"] = last_evac[c]
                            ops["t"].append(t)
                            ia += 1
                            a = dict(kind="evac", cat=cat, k=k, c=c, wait_t=it)
                            if maybe_pad:
                                a["wait_a"], last_dummy_a = last_dummy_a, ia
                            ops["a"].append(a)
                            last_evac[c] = ia
                self.blocks.append((h, r, ops))
                self.cum_v[(h, r)] = iv
                self.cum_a[(h, r)] = ia
                self.cum_g[(h, r)] = ig
                self.cum_t[(h, r)] = it
        self.nv, self.na, self.ng, self.nt = iv, ia, ig, it


def build_tables(plan, sched):
    """Per-core offset tables in exact emission order."""
    qv, qa, qg, qt = [], [], [], []
    for h, r, ops in sched.blocks:
        for op in ops["v"]:
            items = plan.get((h, r, op["cat"]), [])
            k = op["k"]
            if op["kind"] == "meanred":
                qv.append(items[k] * FPI if k < len(items) else DUMMY_G)
            else:
                qv.append(items[k] * FPI if k < len(items) else DUMMY_V)
        for op in ops["g"]:
            if op["kind"] in ("allred", "meanmul"):
                qg.append(0)
            else:
                items = plan.get((h, r, op["cat"]), [])
                k = op["k"]
                qg.append(items[k] * FPI if k < len(items) else DUMMY_G)
        for op in ops["a"]:
            items = plan.get((h, r, op["cat"]), [])
            k = op["k"]
            if op["kind"] == "cpback":
                qa.append(items[k] * FPI if k < len(items) else SCR_OFF[op["scr"]])
            else:  # evac
                qa.append(items[k] * FPI if k < len(items) else DUMMY_A)
        for op in ops["t"]:
            items = plan.get((h, r, op["cat"]), [])
            k = op["k"]
            qt.append(items[k] * FPI if k < len(items) else DUMMY_G)
    return qv, qa, qg, qt


def build_coeffs(plan, sched, samples_core):
    ca = np.ones(max(sched.naff, 1), dtype=np.float32)
    cb = np.zeros(max(sched.naff, 1), dtype=np.float32)
    for h, r, ops in sched.blocks:
        items = plan.get((h, r, "aff"), [])
        for op in ops["v"]:
            if op["kind"] != "aff":
                continue
            k = op["k"]
            if k < len(items):
                t = int(samples_core[r][items[k]])
                ca[op["col"]] = AFF_A[t]
                cb[op["col"]] = AFF_B[t]
    return ca, cb


# ------------------------------------------------------------- program build
def build_program(counts, mins):
    sched = Schedule(counts, mins)
    AFFCLIP, SOLA, POST8 = _get_ops()

    nqv, nqa = max(sched.nv, 1), max(sched.na, 1)
    nqg = max(sched.ng - sched.n_memset, 1)
    nqt = max(sched.nt, 1)
    naff, nmean = max(sched.naff, 1), max(sched.nmean, 1)

    nc = bacc.Bacc()
    x_t = nc.dram_tensor("x", [BL, C, H, W], mybir.dt.float32, kind="ExternalInput")
    qv_t = nc.dram_tensor("qv", [1, nqv], mybir.dt.int32, kind="ExternalInput")
    qa_t = nc.dram_tensor("qa", [1, nqa], mybir.dt.int32, kind="ExternalInput")
    qg_t = nc.dram_tensor("qg", [1, nqg], mybir.dt.int32, kind="ExternalInput")
    qt_t = nc.dram_tensor("qt", [1, nqt], mybir.dt.int32, kind="ExternalInput")
    ca_t = nc.dram_tensor("ca", [P, naff], mybir.dt.float32, kind="ExternalInput")
    cb_t = nc.dram_tensor("cb", [P, naff], mybir.dt.float32, kind="ExternalInput")
    j_t = nc.dram_tensor("jmat", [P, P], mybir.dt.float32, kind="ExternalInput")
    y_t = nc.dram_tensor("y", [BL, C, H, W], mybir.dt.float32, kind="ExternalOutput")

    img = nc.alloc_sbuf_tensor("img", [P, IMG_FREE], mybir.dt.float32).ap()
    qv_s = nc.alloc_sbuf_tensor("qv_s", [1, nqv], mybir.dt.int32).ap()
    qa_s = nc.alloc_sbuf_tensor("qa_s", [1, nqa], mybir.dt.int32).ap()
    qg_s = nc.alloc_sbuf_tensor("qg_s", [1, nqg], mybir.dt.int32).ap()
    qt_s = nc.alloc_sbuf_tensor("qt_s", [1, nqt], mybir.dt.int32).ap()
    ca_s = nc.alloc_sbuf_tensor("ca_s", [P, naff], mybir.dt.float32).ap()
    cb_s = nc.alloc_sbuf_tensor("cb_s", [P, naff], mybir.dt.float32).ap()
    j_s = nc.alloc_sbuf_tensor("j_s", [P, P], mybir.dt.float32).ap()
    cm_s = nc.alloc_sbuf_tensor("cm_s", [P, nmean], mybir.dt.float32).ap()
    red_s = nc.alloc_sbuf_tensor("red_s", [P, nmean], mybir.dt.float32).ap()
    red2_s = nc.alloc_sbuf_tensor("red2_s", [P, nmean], mybir.dt.float32).ap()
    psum = [
        nc.alloc_psum_tensor(f"ps{c}", [P, CHUNK], mybir.dt.float32).ap()
        for c in range(C)
    ]

    NTBL = 7
    tbl = nc.alloc_semaphore("tbl")
    ld = [nc.alloc_semaphore(f"ld{h}") for h in range(HALVES)]
    st = nc.alloc_semaphore("st")
    pv = nc.alloc_semaphore("pv")
    pa = nc.alloc_semaphore("pa")
    pg = nc.alloc_semaphore("pg")
    pt = nc.alloc_semaphore("pt")

    xr = x_t.rearrange("b c h w -> b h c w")
    yr = y_t.rearrange("b c h w -> b h c w")

    def img_view(i):
        return img[:, i * FPI : (i + 1) * FPI].rearrange(
            "p (c s w) -> p c s w", c=C, s=2, w=W
        )

    def dyn_flat(val):
        return img[:, ds(val, FPI)]

    def dyn_view(val):
        return img[:, ds(val, FPI)].rearrange("p (c s w) -> p c s w", c=C, s=2, w=W)

    def vload(eng, ap):
        # value_load minus the runtime-assert instruction: the error
        # notification it emits faults the device under this runtime.
        tmp = eng.alloc_register(f"vl_{nc.next_id()}")
        eng.reg_load(tmp, ap)
        val = eng.snap(tmp, donate=True)
        return nc.s_assert_within(val, 0, MAX_OFF, skip_runtime_assert=True)

    def loadq(eng, qtab, a, n):
        # one batched multi-register load per block instead of n serial loads
        if n == 0:
            return []
        regs = [eng.alloc_register(f"q_{nc.next_id()}") for _ in range(n)]
        eng.reg_load(regs, qtab[0:1, a : a + n])
        return [
            nc.s_assert_within(eng.snap(r, donate=True), 0, MAX_OFF,
                               skip_runtime_assert=True)
            for r in regs
        ]

    def head_waits(eng, h, r, cums):
        eng.wait_ge(ld[h], 16 * 2 * HB)
        if r == 1:
            for sem, cum in cums:
                eng.wait_ge(sem, cum[(h, 0)])

    with contextlib.ExitStack() as exit_ctx, nc.Block() as block:

        @block.sync
        def _(sync):
            for a, b_ in ((qv_s, qv_t), (qa_s, qa_t), (qg_s, qg_t), (qt_s, qt_t),
                          (ca_s, ca_t), (cb_s, cb_t), (j_s, j_t)):
                sync.dma_start(a[:, :], b_[:, :]).then_inc(tbl, 16)
            for h in range(HALVES):
                for i in range(h * HB, (h + 1) * HB):
                    v = img_view(i)
                    sync.dma_start(v[:, :, 0, :], xr[i, 0:P, :, :]).then_inc(
                        ld[h], 16
                    )
                    sync.dma_start(v[:, :, 1, :], xr[i, P:H, :, :]).then_inc(
                        ld[h], 16
                    )
            for h in range(HALVES):
                sync.wait_ge(pv, sched.cum_v[(h, 1)])
                sync.wait_ge(pa, sched.cum_a[(h, 1)])
                sync.wait_ge(pg, sched.cum_g[(h, 1)])
                for i in range(h * HB, (h + 1) * HB):
                    v = img_view(i)
                    sync.dma_start(yr[i, 0:P, :, :], v[:, :, 0, :]).then_inc(st, 16)
                    sync.dma_start(yr[i, P:H, :, :], v[:, :, 1, :]).then_inc(st, 16)
            sync.wait_ge(st, 16 * 2 * BL)

        @block.vector
        def _(vector):
            vector.wait_ge(tbl, 16 * NTBL)
            vector.wait_ge(pg, sched.n_memset)
            qi = 0
            for h, r, ops in sched.blocks:
                if not ops["v"]:
                    continue
                head_waits(vector, h, r,
                           [(pa, sched.cum_a), (pg, sched.cum_g),
                            (pv, sched.cum_v), (pt, sched.cum_t)])
                offs = loadq(vector, qv_s, qi, len(ops["v"]))
                for j, op in enumerate(ops["v"]):
                    if op.get("wait_v") is not None:
                        vector.wait_ge(pv, op["wait_v"])
                    src = dyn_flat(offs[j])
                    if op["kind"] == "meanred":
                        vector.tensor_reduce(
                            red_s[:, op["col"] : op["col"] + 1], src,
                            mybir.AxisListType.X, mybir.AluOpType.add,
                        ).then_inc(pv, 1)
                    elif op["kind"] == "caff":
                        vector.wait_ge(pg, op["wait_g"])
                        vector._custom_dve(
                            AFFCLIP, out=src, in0=src, s0=1.5,
                            s1=cm_s[:, op["col"] : op["col"] + 1],
                        ).then_inc(pv, 1)
                    elif op["kind"] == "aff":
                        vector._custom_dve(
                            AFFCLIP, out=src, in0=src,
                            s0=ca_s[:, op["col"] : op["col"] + 1],
                            s1=cb_s[:, op["col"] : op["col"] + 1],
                        ).then_inc(pv, 1)
                    elif op["kind"] == "sol":
                        vector._custom_dve(SOLA, out=src, in0=src).then_inc(pv, 1)
                    else:
                        vector._custom_dve(
                            POST8, out=src, in0=src, s0=0.25, s1=0.5, imm2=0.75
                        ).then_inc(pv, 1)
                qi += len(ops["v"])

        @block.gpsimd
        def _(gpsimd):
            for off in (DUMMY_V, DUMMY_G, DUMMY_A, *SCR_OFF):
                gpsimd.memset(img[:, off : off + FPI], 0.0).then_inc(pg, 1)
            gpsimd.wait_ge(tbl, 16 * NTBL)
            gpsimd.wait_ge(pg, sched.n_memset)
            qi = 0
            for h, r, ops in sched.blocks:
                if not ops["g"]:
                    continue
                head_waits(gpsimd, h, r,
                           [(pa, sched.cum_a), (pv, sched.cum_v),
                            (pg, sched.cum_g), (pt, sched.cum_t)])
                offs = loadq(gpsimd, qg_s, qi, len(ops["g"]))
                for j, op in enumerate(ops["g"]):
                    if op["kind"] == "allred":
                        gpsimd.wait_ge(pv, op["wait_v"])
                        gpsimd.partition_all_reduce(
                            red2_s[:, op["col"] : op["col"] + 1],
                            red_s[:, op["col"] : op["col"] + 1],
                            channels=P, reduce_op=bass_isa.ReduceOp.add,
                        ).then_inc(pg, 1)
                    elif op["kind"] == "meanmul":
                        gpsimd.wait_ge(pg, op["wait_g"])
                        gpsimd.tensor_scalar_mul(
                            cm_s[:, op["col"] : op["col"] + 1],
                            red2_s[:, op["col"] : op["col"] + 1],
                            MEAN_SCALE,
                        ).then_inc(pg, 1)
                    else:  # cpout (flip_w): w-reversed copy into scratch
                        if "wait_a" in op:
                            gpsimd.wait_ge(pa, op["wait_a"])
                        v = dyn_view(offs[j])
                        dst = img[
                            :, SCR_OFF[op["scr"]] : SCR_OFF[op["scr"]] + FPI
                        ].rearrange("p (c s w) -> p c s w", c=C, s=2, w=W)
                        gpsimd.tensor_copy(dst, v[:, :, :, ::-1]).then_inc(pg, 1)
                qi += len(ops["g"])

        @block.tensor
        def _(tensor):
            tensor.wait_ge(tbl, 16 * NTBL)
            tensor.wait_ge(pg, sched.n_memset)
            qi = 0
            for h, r, ops in sched.blocks:
                if not ops["t"]:
                    continue
                head_waits(tensor, h, r,
                           [(pa, sched.cum_a), (pv, sched.cum_v),
                            (pg, sched.cum_g), (pt, sched.cum_t)])
                offs = loadq(tensor, qt_s, qi, len(ops["t"]))
                for j, op in enumerate(ops["t"]):
                    if "wait_a" in op:
                        tensor.wait_ge(pa, op["wait_a"])
                    c = op["c"]
                    rhs = dyn_flat(offs[j])[:, c * CHUNK : (c + 1) * CHUNK]
                    tensor.matmul(
                        psum[c][:, :], j_s[:, :], rhs, start=True, stop=True
                    ).then_inc(pt, 1)
                qi += len(ops["t"])

        @block.scalar
        def _(scalar):
            scalar.wait_ge(tbl, 16 * NTBL)
            scalar.wait_ge(pg, sched.n_memset)
            qi = 0
            for h, r, ops in sched.blocks:
                if not ops["a"]:
                    continue
                head_waits(scalar, h, r,
                           [(pv, sched.cum_v), (pg, sched.cum_g),
                            (pa, sched.cum_a), (pt, sched.cum_t)])
                offs = loadq(scalar, qa_s, qi, len(ops["a"]))
                for j, op in enumerate(ops["a"]):
                    if op.get("wait_a") is not None:
                        scalar.wait_ge(pa, op["wait_a"])
                    if op["kind"] == "cpback":
                        scalar.wait_ge(pg, op["wait_g"])
                        s = img[:, SCR_OFF[op["scr"]] : SCR_OFF[op["scr"]] + FPI]
                        scalar.activation(
                            dyn_flat(offs[j]), s, mybir.ActivationFunctionType.Copy
                        ).then_inc(pa, 1)
                    else:  # evac: psum chunk -> home slot with s-swap (+w-rev)
                        scalar.wait_ge(pt, op["wait_t"])
                        c = op["c"]
                        pview = psum[c].rearrange("p (s w) -> p s w", s=2, w=W)
                        out = dyn_view(offs[j])[:, c]
                        out = out[:, ::-1, ::-1] if op["cat"] == "flip_sw" \
                            else out[:, ::-1, :]
                        scalar.activation(
                            out, pview, mybir.ActivationFunctionType.Copy
                        ).then_inc(pa, 1)
                qi += len(ops["a"])

    nc.compile()
    return nc, sched


# ---------------------------------------------------------------- entrypoint
def _prepare(samples):
    samples = np.asarray(samples)
    plans = [plan_core(samples[:, c * BL : (c + 1) * BL]) for c in range(NCORES)]
    counts, mins = {}, {}
    for h in range(HALVES):
        for r in range(NSTEP):
            for cat in ALL_CATS:
                ns = [len(p.get((h, r, cat), [])) for p in plans]
                counts[(h, r, cat)] = max(ns)
                mins[(h, r, cat)] = min(ns)
    return plans, counts, mins


def make_in_maps(x, samples, plans, sched):
    samples = np.asarray(samples)
    jmat = np.eye(P, dtype=np.float32)[::-1].copy()

    def tab(lst):
        return (np.asarray(lst, np.int32).reshape(1, -1)
                if lst else np.zeros((1, 1), np.int32))

    in_maps = []
    for c in range(NCORES):
        qv, qa, qg, qt = build_tables(plans[c], sched)
        ca, cb = build_coeffs(plans[c], sched, samples[:, c * BL : (c + 1) * BL])
        in_maps.append(
            dict(
                x=np.ascontiguousarray(x[c * BL : (c + 1) * BL]),
                qv=tab(qv), qa=tab(qa), qg=tab(qg), qt=tab(qt),
                ca=np.tile(ca.reshape(1, -1), (P, 1)),
                cb=np.tile(cb.reshape(1, -1), (P, 1)),
                jmat=jmat,
            )
        )
    return in_maps


def kernel(x, prob, samples):
    from concourse.bass_utils import run_bass_kernel_spmd

    x = np.ascontiguousarray(np.asarray(x, dtype=np.float32))
    plans, counts, mins = _prepare(samples)
    nc, sched = build_program(counts, mins)
    in_maps = make_in_maps(x, samples, plans, sched)
    res = run_bass_kernel_spmd(nc, in_maps, core_ids=list(range(NCORES)))
    out = np.concatenate([res.results[c]["y"] for c in range(NCORES)], axis=0)
    return out.reshape(B, C, H, W).astype(np.float32)
